# revision 14
# baseline (speedup 1.0000x reference)
"""Trainium2 Bass kernel for channel attention (XCA-style) nn.Module.

Pipeline per image (one image per NeuronCore, batch=8 over 8 cores):
  qkv 1x1 conv (matmul) -> 3x3 depthwise conv -> l2norm channel attention
  -> (attn @ v folded with proj 1x1 into a single matmul).

Key algebraic restructurings:
  * Gram matrix q@k^T is accumulated UNNORMALIZED over pixel stripes in PSUM;
    l2 normalization is applied afterwards as row/col scaling (norms come from
    the gram diagonal for free).
  * attn@v followed by the 1x1 proj collapses into y = M @ v with
    M^T[48h+d, :] = A_h^T @ proj_w^T[48h:48h+48, :]  (tiny per-head matmuls).
  * Depthwise conv = 9 fused multiply-accumulate passes over flat-shifted
    views (scalar_tensor_tensor with per-partition tap weights), plus tiny
    strided border-fix passes for the x-wrap columns.
  * Channels are stored head-interleaved [q_h0,k_h0,q_h1,k_h1,...,v] so the
    per-head gram operand is one contiguous column slice after transpose.
"""

import numpy as np
import ml_dtypes

import concourse.bass as bass
import concourse.tile as tile
from concourse import mybir, bacc
from concourse.bass_utils import run_bass_kernel_spmd

F32 = mybir.dt.float32
BF16 = mybir.dt.bfloat16
AX = mybir.AxisListType
OP = mybir.AluOpType
ACTF = mybir.ActivationFunctionType

C, H, W = 192, 128, 128
HW = H * W
HEADS, CH = 4, 48
TAPS = [(dy, dx) for dy in (-1, 0, 1) for dx in (-1, 0, 1)]
SA = 16   # pass-A stripe rows (q,k)
SB = 16   # pass-B stripe rows (v)
# taps computed on the PE as diagonal matmuls (by tap index); rest on DVE
PE_TAPS = ()

_cached = {}


def _build_program():
    nc = bacc.Bacc("TRN2", target_bir_lowering=False, debug=False, num_devices=8)

    x_d = nc.dram_tensor("x", [C, HW], F32, kind="ExternalInput").ap()
    w1t_d = nc.dram_tensor("w1t", [C, 576], BF16, kind="ExternalInput").ap()
    dwt_d = nc.dram_tensor("dwt", [128, 5, 18], F32, kind="ExternalInput").ap()
    pwt_d = nc.dram_tensor("pwt", [48, 4, C], BF16, kind="ExternalInput").ap()
    i96_d = nc.dram_tensor("i96", [96, 96], F32, kind="ExternalInput").ap()
    i128_d = nc.dram_tensor("i128", [128, 128], BF16, kind="ExternalInput").ap()
    t4_d = nc.dram_tensor("t4", [1, 4], F32, kind="ExternalInput").ap()
    y_d = nc.dram_tensor("y", [C, HW], F32, kind="ExternalOutput").ap()

    with tile.TileContext(nc) as tc:
        _emit(nc, tc, x_d, w1t_d, dwt_d, pwt_d, i96_d, i128_d, t4_d, y_d)
    nc.finalize()
    return nc


def _emit(nc, tc, x_d, w1t_d, dwt_d, pwt_d, i96_d, i128_d, t4_d, y_d):
    from contextlib import ExitStack

    with ExitStack() as top:
        persist = top.enter_context(tc.tile_pool(name="persist", bufs=1))

        # ---- persistent weights / constants ----
        w1t0 = persist.tile([128, 576], BF16)
        w1t1 = persist.tile([64, 576], BF16)
        nc.sync.dma_start(out=w1t0, in_=w1t_d[0:128, :])
        nc.sync.dma_start(out=w1t1, in_=w1t_d[128:192, :])
        dwt = persist.tile([128, 5, 18], F32)
        nc.sync.dma_start(out=dwt, in_=dwt_d)
        pwt = persist.tile([48, 4, C], BF16)
        nc.sync.dma_start(out=pwt, in_=pwt_d)
        i96 = persist.tile([96, 96], F32)
        nc.sync.dma_start(out=i96, in_=i96_d)
        i128 = persist.tile([128, 128], BF16)
        nc.sync.dma_start(out=i128, in_=i128_d)
        # per-(chunk, corner-tap) diagonal weight matrices for PE conv taps
        diag = persist.tile([128, 20, 128], BF16)
        for ci in range(5):
            for j, t in enumerate([0, 2, 6, 8]):
                nc.vector.tensor_scalar_mul(diag[:, ci * 4 + j, :], i128,
                                            dwt[:, ci, t:t + 1])
        t4s = persist.tile([1, 4], F32)
        nc.sync.dma_start(out=t4s, in_=t4_d)

        # persistent bf16 copy of x
        xb0 = persist.tile([128, HW], BF16)
        xb1 = persist.tile([64, HW], BF16)

        # results of pass A / finalize, used by pass B
        mta = persist.tile([128, C], BF16)
        mtb = persist.tile([64, C], BF16)

        gpool = top.enter_context(tc.tile_pool(name="gpool", bufs=1, space="PSUM"))
        g_ps = gpool.tile([96, HEADS, 96], F32)

        # ================= pass A: q,k =================
        with ExitStack() as pa:
            xf_p = pa.enter_context(tc.tile_pool(name="xf_p", bufs=2))
            pre_p = pa.enter_context(tc.tile_pool(name="pre_p", bufs=2))
            qs_p = pa.enter_context(tc.tile_pool(name="qs_p", bufs=2))
            acc_p = pa.enter_context(tc.tile_pool(name="acc_p", bufs=2))
            qkt_p = pa.enter_context(tc.tile_pool(name="qkt_p", bufs=2))
            ps_p = pa.enter_context(tc.tile_pool(name="ps_p", bufs=2, space="PSUM"))
            cps_p = pa.enter_context(tc.tile_pool(name="cps_p", bufs=3, space="PSUM"))
            tps_p = pa.enter_context(tc.tile_pool(name="tps_p", bufs=2, space="PSUM"))

            def cast_stripe(s):
                px = slice(s * SA * W, (s + 1) * SA * W)
                xf0 = xf_p.tile([128, SA * W], F32, tag="xf")
                xf1 = xf_p.tile([64, SA * W], F32, tag="xf")
                nc.sync.dma_start(out=xf0, in_=x_d[0:128, px])
                nc.sync.dma_start(out=xf1, in_=x_d[128:192, px])
                nc.vector.tensor_copy(xb0[:, px], xf0)
                nc.vector.tensor_copy(xb1[:, px], xf1)

            for s in range(H // SA + 1):
                if s < H // SA:
                    cast_stripe(s)
                if s >= 1:
                    _conv_stripe_qk(nc, s - 1, SA, w1t0, w1t1, xb0, xb1,
                                    dwt, diag, i128,
                                    pre_p, qs_p, acc_p, qkt_p, ps_p, cps_p,
                                    tps_p, g_ps, n_stripes=H // SA)

        # ================= finalize attention -> MT =================
        with ExitStack() as fz:
            fpool = fz.enter_context(tc.tile_pool(name="fpool", bufs=1))
            fps = fz.enter_context(tc.tile_pool(name="fps", bufs=2, space="PSUM"))

            gs = fpool.tile([96, HEADS, 96], F32)
            nc.scalar.copy(gs, g_ps)

            # norms^2 from the gram diagonal
            i96b = bass.AP(tensor=i96.tensor, offset=i96.offset,
                           ap=[list(i96.ap[0]), [0, HEADS], [1, 96]])
            gdiag = fpool.tile([96, HEADS, 96], F32)
            nc.vector.tensor_mul(gdiag, gs, i96b)
            nrm2 = fpool.tile([96, HEADS], F32)
            nc.vector.reduce_sum(nrm2, gdiag, axis=AX.X)
            nrm = fpool.tile([96, HEADS], F32)
            nc.scalar.activation(nrm, nrm2, ACTF.Sqrt)
            nc.vector.tensor_scalar_max(nrm, nrm, 1e-12)
            rstd = fpool.tile([96, HEADS], F32)
            nc.vector.reciprocal(rstd, nrm)

            # q-side scale = rstd_q * temperature[h]
            t4b = fpool.tile([48, HEADS], F32)
            nc.sync.dma_start(
                out=t4b,
                in_=bass.AP(tensor=t4_d.tensor, offset=t4_d.offset,
                            ap=[[0, 48], [1, HEADS]]))
            rq = fpool.tile([48, HEADS], F32)
            nc.vector.tensor_mul(rq, rstd[0:48, :], t4b)

            # k-side scale as a broadcast row: transpose rstd[48:96] -> [4, 48]
            rkk = fpool.tile([48, HEADS], F32)
            nc.sync.dma_start(out=rkk, in_=rstd[48:96, :])
            rkps = fps.tile([4, 48], F32)
            nc.tensor.transpose(rkps, rkk, i96[0:48, 0:48])
            rkrow = fpool.tile([4, 48], F32)
            nc.vector.tensor_copy(rkrow, rkps)
            dram_p = fz.enter_context(tc.tile_pool(name="dram_p", bufs=1,
                                                   space="DRAM"))
            rkd = dram_p.tile([4, 48], F32)
            nc.sync.dma_start(out=rkd, in_=rkrow)
            rk = fpool.tile([48, HEADS, 48], F32)
            for h in range(HEADS):
                bsrc = bass.AP(tensor=rkd.tensor,
                               offset=rkd.offset + h * 48,
                               ap=[[0, 48], [1, 48]])
                nc.sync.dma_start(out=rk[:, h, :], in_=bsrc)

            # scaled logits Z, then softmax rows
            z = fpool.tile([48, HEADS, 48], F32)
            for h in range(HEADS):
                nc.vector.scalar_tensor_tensor(
                    out=z[:, h, :], in0=gs[0:48, h, 48:96],
                    scalar=rq[:, h:h + 1], in1=rk[:, h, :],
                    op0=OP.mult, op1=OP.mult)
            mx = fpool.tile([48, HEADS], F32)
            nc.vector.reduce_max(mx, z, axis=AX.X)
            nmx = fpool.tile([48, HEADS], F32)
            nc.vector.tensor_scalar_mul(nmx, mx, -1.0)
            ez = fpool.tile([48, HEADS, 48], F32)
            for h in range(HEADS):
                nc.scalar.activation(ez[:, h, :], z[:, h, :], ACTF.Exp,
                                     bias=nmx[:, h:h + 1], scale=1.0)
            sm = fpool.tile([48, HEADS], F32)
            nc.vector.reduce_sum(sm, ez, axis=AX.X)
            rs = fpool.tile([48, HEADS], F32)
            nc.vector.reciprocal(rs, sm)
            a_bf = fpool.tile([48, HEADS, 48], BF16)
            for h in range(HEADS):
                nc.vector.tensor_scalar_mul(a_bf[:, h, :], ez[:, h, :],
                                            rs[:, h:h + 1])

            # M^T_h = A_h^T @ proj_w^T[48h:48h+48, :]
            m_bf = fpool.tile([48, HEADS, C], BF16)
            for h in range(HEADS):
                mps = fps.tile([48, C], F32, tag="mps")
                nc.tensor.matmul(mps, a_bf[:, h, :], pwt[:, h, :],
                                 start=True, stop=True)
                nc.scalar.copy(m_bf[:, h, :], mps)

            # assemble MT tiles (partition-moving SBUF->SBUF DMAs)
            nc.sync.dma_start(out=mta[0:48, :], in_=m_bf[:, 0, :])
            nc.sync.dma_start(out=mta[48:96, :], in_=m_bf[:, 1, :])
            nc.sync.dma_start(out=mta[96:128, :], in_=m_bf[0:32, 2, :])
            nc.sync.dma_start(out=mtb[0:16, :], in_=m_bf[32:48, 2, :])
            nc.sync.dma_start(out=mtb[16:64, :], in_=m_bf[:, 3, :])

        # ================= pass B: v + output =================
        with ExitStack() as pb:
            pre_p = pb.enter_context(tc.tile_pool(name="preb_p", bufs=2))
            qs_p = pb.enter_context(tc.tile_pool(name="qsb_p", bufs=2))
            acc_p = pb.enter_context(tc.tile_pool(name="accb_p", bufs=2))
            y_p = pb.enter_context(tc.tile_pool(name="y_p", bufs=3))
            ps_p = pb.enter_context(tc.tile_pool(name="psb_p", bufs=2, space="PSUM"))
            cps_p = pb.enter_context(tc.tile_pool(name="cpsb_p", bufs=3, space="PSUM"))
            yps_p = pb.enter_context(tc.tile_pool(name="yps_p", bufs=2, space="PSUM"))

            for s in range(H // SB):
                _conv_stripe_v(nc, s, SB, w1t0, w1t1, xb0, xb1, dwt, diag,
                               pre_p, qs_p, acc_p, ps_p, cps_p, yps_p, y_p,
                               mta, mtb, y_d, n_stripes=H // SB)


def _dw_conv(nc, pre, qs, acc, dwt, diag, oc_list, S, cps_p):
    """9-tap depthwise conv on padded stripe tiles ([p, 2+(S+2)*W+2]).
    Corner taps (dy,dx)=(+-1,+-1) run on the PE as diagonal matmuls
    accumulated in PSUM; the remaining 5 run on the DVE, the first one
    folding the PSUM partial in via scalar_tensor_tensor."""
    base = 2 + W
    n = S * W
    PE_T = [0, 2, 6, 8]          # corner tap indices (dy+1)*3+(dx+1)
    DVE_T = [3, 4, 5, 7]         # (-1,0) handled by the combine pass
    for i, oc in enumerate(oc_list):
        p, q, a = pre[i], qs[i], acc[i]
        np_ = p.shape[0]
        # PE corner taps -> PSUM, 512 px at a time
        cps = []
        for ntl in range(n // 512):
            cp = cps_p.tile([np_, 512], F32, tag="cps", name="cps")
            for j, t in enumerate(PE_T):
                dy, dx = TAPS[t]
                off = base + dy * W + dx + 512 * ntl
                nc.tensor.matmul(
                    cp, diag[:np_, oc * 4 + j, :np_], p[:, off:off + 512],
                    start=(j == 0), stop=(j == len(PE_T) - 1))
            cps.append(cp)
        # DVE: fold PSUM + tap (-1,0), then chain the remaining taps
        wv = dwt[:np_, oc, 1:2]
        for ntl in range(n // 512):
            off = base - W + 512 * ntl
            nc.vector.scalar_tensor_tensor(
                out=a[:, 512 * ntl:512 * (ntl + 1)],
                in0=p[:, off:off + 512], scalar=wv, in1=cps[ntl],
                op0=OP.mult, op1=OP.add)
        for t in DVE_T:
            dy, dx = TAPS[t]
            wv = dwt[:np_, oc, t:t + 1]
            if dx == 0:
                v = p[:, base + dy * W: base + dy * W + n]
            elif dx == 1:
                v = q[:, base + dy * W: base + dy * W + n]
            else:
                v = q[:, base + dy * W - 2: base + dy * W - 2 + n]
            nc.vector.scalar_tensor_tensor(
                out=a, in0=v, scalar=wv, in1=a, op0=OP.mult, op1=OP.add)
        # x-border fixes: subtract the wrapped columns (cols 9.. hold -w)
        for dy in (-1, 0, 1):
            wv = dwt[:np_, oc, 9 + (dy + 1) * 3: 9 + (dy + 1) * 3 + 1]
            junk = p[:, base + dy * W - 1: base + dy * W - 1 + (S - 1) * W + 1: W]
            outv = a[:, 0: (S - 1) * W + 1: W]
            nc.vector.scalar_tensor_tensor(
                out=outv, in0=junk, scalar=wv, in1=outv,
                op0=OP.mult, op1=OP.add)
            wv2 = dwt[:np_, oc, 9 + (dy + 1) * 3 + 2: 9 + (dy + 1) * 3 + 3]
            junk2 = p[:, base + dy * W + W: base + dy * W + W + (S - 1) * W + 1: W]
            outv2 = a[:, W - 1: W - 1 + (S - 1) * W + 1: W]
            nc.vector.scalar_tensor_tensor(
                out=outv2, in0=junk2, scalar=wv2, in1=outv2,
                op0=OP.mult, op1=OP.add)


def _stripe_matmul(nc, s, S, n_stripes, oc_defs, w1t0, w1t1, xb0, xb1,
                   pre, ps_p):
    """1x1-conv matmul into padded stripe tiles pre (incl. halo rows),
    memsetting pads and out-of-image halo rows."""
    r0 = s * S
    T = (S + 2) * W
    lo, hi = max(r0 - 1, 0), min(r0 + S + 1, H)
    for i, (ocp, ocsl) in enumerate(oc_defs):
        p = pre[i]
        if s < 2:
            nc.gpsimd.memset(p[:, 0:2], 0.0)
            nc.gpsimd.memset(p[:, 2 + T: 4 + T], 0.0)
        if s == 0:
            nc.gpsimd.memset(p[:, 2: 2 + W], 0.0)
        if s == n_stripes - 1:
            nc.gpsimd.memset(p[:, 2 + T - W: 2 + T], 0.0)
        # valid data region, tiled by <=512 columns
        vlo, vhi = lo * W, hi * W
        pos = vlo
        while pos < vhi:
            nt = min(512, vhi - pos)
            ps = ps_p.tile([ocp, 512], F32, tag="mmps")
            px = slice(pos, pos + nt)
            nc.tensor.matmul(ps[:, 0:nt], w1t0[:, ocsl], xb0[:, px],
                             start=True, stop=False)
            nc.tensor.matmul(ps[:, 0:nt], w1t1[:, ocsl], xb1[:, px],
                             start=False, stop=True)
            dst = p[:, 2 + (pos - (r0 - 1) * W): 2 + (pos - (r0 - 1) * W) + nt]
            nc.scalar.copy(dst, ps[:, 0:nt])
            pos += nt


def _conv_stripe_qk(nc, s, S, w1t0, w1t1, xb0, xb1, dwt, diag, i128,
                    pre_p, qs_p, acc_p, qkt_p, ps_p, cps_p, tps_p, g_ps,
                    n_stripes):
    T = (S + 2) * W
    pre = [pre_p.tile([128, T + 4], BF16, tag=f"pre{i}", name=f"pre{i}") for i in range(3)]
    oc_defs = [(128, slice(0, 128)), (128, slice(128, 256)), (128, slice(256, 384))]
    _stripe_matmul(nc, s, S, n_stripes, oc_defs, w1t0, w1t1, xb0, xb1, pre, ps_p)

    qs = [qs_p.tile([128, T + 4], BF16, tag=f"qs{i}", name=f"qs{i}") for i in range(3)]
    for i in range(3):
        nc.sync.dma_start(out=qs[i][:, 0:T + 3], in_=pre[i][:, 1:T + 4])

    acc = [acc_p.tile([128, S * W], BF16, tag=f"acc{i}", name=f"acc{i}") for i in range(3)]
    _dw_conv(nc, pre, qs, acc, dwt, diag, [0, 1, 2], S, cps_p)

    # transpose to [px, ch] on the PE (packs of 4 into one PSUM bank)
    qkt = qkt_p.tile([128, S * W // 128, 384], BF16)
    for i in range(3):
        for g in range(S * W // 512):
            tps = tps_p.tile([128, 4, 128], BF16, tag="tps", name="tps")
            for k in range(4):
                nc.tensor.transpose(
                    tps[:, k, :], acc[i][:, (4 * g + k) * 128:(4 * g + k + 1) * 128],
                    i128)
            nc.scalar.copy(qkt[:, 4 * g:4 * g + 4, 128 * i:128 * (i + 1)], tps)
    first = (s == 0)
    last = (s == n_stripes - 1)
    for pc in range(S * W // 128):
        for h in range(HEADS):
            nc.tensor.matmul(
                g_ps[:, h, :], qkt[:, pc, 96 * h:96 * h + 96],
                qkt[:, pc, 96 * h:96 * h + 96],
                start=(first and pc == 0),
                stop=(last and pc == S * W // 128 - 1),
                skip_group_check=True)


def _conv_stripe_v(nc, s, S, w1t0, w1t1, xb0, xb1, dwt, diag,
                   pre_p, qs_p, acc_p, ps_p, cps_p, yps_p, y_p, mta, mtb, y_d,
                   n_stripes):
    T = (S + 2) * W
    pre = [pre_p.tile([128, T + 4], BF16, tag="prev0", name="prev0"),
           pre_p.tile([64, T + 4], BF16, tag="prev1", name="prev1")]
    oc_defs = [(128, slice(384, 512)), (64, slice(512, 576))]
    _stripe_matmul(nc, s, S, n_stripes, oc_defs, w1t0, w1t1, xb0, xb1, pre, ps_p)

    qs = [qs_p.tile([128, T + 4], BF16, tag="qsv0", name="qsv0"),
          qs_p.tile([64, T + 4], BF16, tag="qsv1", name="qsv1")]
    for i in range(2):
        nc.sync.dma_start(out=qs[i][:, 0:T + 3], in_=pre[i][:, 1:T + 4])

    acc = [acc_p.tile([128, S * W], BF16, tag="accv0", name="accv0"),
           acc_p.tile([64, S * W], BF16, tag="accv1", name="accv1")]
    _dw_conv(nc, pre, qs, acc, dwt, diag, [3, 4], S, cps_p)

    # y = M^T.T @ v  (attn+proj folded), evacuate f32 and store per chunk
    r0 = s * S
    for nt in range(S * W // 512):
        px = slice(512 * nt, 512 * (nt + 1))
        dpx = slice(r0 * W + 512 * nt, r0 * W + 512 * (nt + 1))
        yp0 = yps_p.tile([128, 512], F32, tag="yp")
        nc.tensor.matmul(yp0, mta[:, 0:128], acc[0][:, px], start=True, stop=False)
        nc.tensor.matmul(yp0, mtb[:, 0:128], acc[1][:, px], start=False, stop=True)
        y0 = y_p.tile([128, 512], F32, tag="y0")
        nc.scalar.copy(y0, yp0)
        nc.sync.dma_start(out=y_d[0:128, dpx], in_=y0)
        yp1 = yps_p.tile([64, 512], F32, tag="yp")
        nc.tensor.matmul(yp1, mta[:, 128:192], acc[0][:, px], start=True, stop=False)
        nc.tensor.matmul(yp1, mtb[:, 128:192], acc[1][:, px], start=False, stop=True)
        y1 = y_p.tile([64, 512], F32, tag="y1")
        nc.scalar.copy(y1, yp1)
        nc.sync.dma_start(out=y_d[128:192, dpx], in_=y1)


# ---------------- host glue ----------------

def _host_inputs(x, qkv_w, dw_w, proj_w, temperature):
    perm = []
    for h in range(HEADS):
        perm += list(range(h * CH, (h + 1) * CH))
        perm += list(range(C + h * CH, C + (h + 1) * CH))
    perm += list(range(2 * C, 3 * C))
    perm = np.array(perm)

    w1 = np.asarray(qkv_w)[perm]                       # (576, 192)
    w1t = np.ascontiguousarray(w1.T).astype(ml_dtypes.bfloat16)
    dw = np.asarray(dw_w)[perm, 0]                     # (576, 3, 3)
    dwt = np.zeros((128, 5, 18), np.float32)
    for ci in range(5):
        rows = min(128, 576 - ci * 128)
        taps = dw[ci * 128: ci * 128 + rows].reshape(rows, 9)
        dwt[:rows, ci, 0:9] = taps
        dwt[:rows, ci, 9:18] = -taps
    # pwt[p, h, o] = proj_w.T[48h + p, o]
    pT = np.asarray(proj_w).T.astype(np.float32)       # (192 c, 192 o)
    pwt = np.stack([pT[48 * h:48 * (h + 1)] for h in range(4)],
                   axis=1).astype(ml_dtypes.bfloat16)
    i96 = np.eye(96, dtype=np.float32)
    i128 = np.eye(128, dtype=ml_dtypes.bfloat16)
    t4 = np.asarray(temperature).reshape(1, HEADS).astype(np.float32)
    shared = {
        "w1t": w1t, "dwt": dwt, "pwt": pwt, "i96": i96, "i128": i128, "t4": t4,
    }
    xs = np.asarray(x).reshape(8, C, HW).astype(np.float32)
    return shared, xs


def kernel(x, qkv_w, dw_w, proj_w, temperature, _trace=False):
    if "nc" not in _cached:
        _cached["nc"] = _build_program()
    nc = _cached["nc"]
    shared, xs = _host_inputs(x, qkv_w, dw_w, proj_w, temperature)
    in_maps = [dict(shared, x=np.ascontiguousarray(xs[i])) for i in range(8)]
    res = run_bass_kernel_spmd(nc, in_maps, core_ids=list(range(8)),
                               trace=_trace)
    out = np.stack([np.asarray(res.results[i]["y"]).reshape(C, H, W)
                    for i in range(8)])
    if _trace:
        _cached["last_exec_time_ns"] = res.exec_time_ns
        _cached["last_results"] = res
    return out
